# revision 22
# baseline (speedup 1.0000x reference)
#!/usr/bin/env python3
"""GroupedQueryAttention Trainium2 kernel, tensor-parallel over heads on 8
NeuronCores.

Reference model: B=2, S=2048, H=4096, NH=32 query heads, NKV=8 kv heads,
HD=128, RoPE base 5e5, softmax attention, o-proj.

Sharding: core c owns kv head c and query heads 4c..4c+3 (groups stay
aligned).  Wq/Wo sharded by query head, Wk/Wv by kv head.  Each core
computes a rank-512 slice of the o-proj contraction; the host sums the 8
partial outputs (the all-reduce of row-parallel TP done on host at gather
time).

v4 design notes (on top of v3):
  * Weight loads split into 16 just-in-time pieces on the scalar queue so
    the first projection matmul starts ~5us in instead of ~28us; cos/sin/
    tables/wo trickle in behind the x stream.
  * Chunk 3's Q projection is deferred out of the P phase into the A loop
    as PE filler for the exp-latency-bound first attention units (its x
    tiles are re-streamed per head on the idle sync queue).  Combined with
    o-proj units carried across the batch boundary, the PE no longer
    idles (and HAM no longer drops to half clock) at phase transitions.
  * Softmax denominator: the per-head tree is non-destructive and its
    first level issues mid-unit; the cross-partition sum uses an all-ones
    [128,128] stationary so ONE matmul yields the denominator already
    broadcast to 128 partitions (the old per-head bcsel broadcast matmuls
    and their PSUM bank are gone -- that bank now feeds the Q filler).
    reciprocal_approx_fast (single custom-DVE op, ~18 bits) replaces the
    slow iterative reciprocal; normalize happens per head.
"""
import sys

for _p in ("/opt/trn_rl_repo",):
    if _p not in sys.path:
        sys.path.insert(0, _p)

import numpy as np

import concourse.bacc as bacc
import concourse.mybir as mybir
from concourse import tile
from concourse.bass_utils import run_bass_kernel_spmd

B, S, H = 2, 2048, 4096
NH, NKV, HD = 32, 8, 128
NCORES = 8
QH_PER_CORE = NH // NCORES          # 4 query heads / core
QD = QH_PER_CORE * HD               # 512 q dims / core
ROPE_BASE = 500000.0
T = B * S                           # 4096 tokens
TCH = 512                           # token chunk (proj N, attn i-chunk)
NTCH = S // TCH                     # 4 chunks per batch
HT = H // 128                       # 32 h-tiles
JT = S // 128                       # 16 j-tiles per batch
NHCH = H // TCH                     # 8 o-proj column chunks
SCALE = 1.0 / np.sqrt(HD)

F32 = mybir.dt.float32
BF16 = mybir.dt.bfloat16


def _build_nc():
    nc = bacc.Bacc("TRN2", target_bir_lowering=False, debug=False)
    xt = nc.dram_tensor("xt", [H, T], BF16, kind="ExternalInput").ap()
    wq = nc.dram_tensor("wq", [H, QD], BF16, kind="ExternalInput").ap()
    wk = nc.dram_tensor("wk", [H, HD], BF16, kind="ExternalInput").ap()
    wv = nc.dram_tensor("wv", [H, HD], BF16, kind="ExternalInput").ap()
    wo = nc.dram_tensor("wo", [QD, H], BF16, kind="ExternalInput").ap()
    cosx = nc.dram_tensor("cosx", [HD, S], BF16, kind="ExternalInput").ap()
    ssin = nc.dram_tensor("ssin", [HD, S], BF16, kind="ExternalInput").ap()
    ident = nc.dram_tensor("ident", [128, 128], BF16, kind="ExternalInput").ap()
    # all-ones stationary: one matmul = partition-sum broadcast to all rows
    ones = nc.dram_tensor("ones", [128, 128], BF16, kind="ExternalInput").ap()
    out = nc.dram_tensor("out_part", [T, H], BF16, kind="ExternalOutput").ap()

    with tile.TileContext(nc) as tc, \
         nc.allow_low_precision(reason="bf16 matmuls; bf16 attn probs"):
        with tc.tile_pool(name="persist", bufs=1) as persist, \
             tc.tile_pool(name="store", bufs=1) as spool, \
             tc.tile_pool(name="ap", bufs=2) as apool:
            # ---- resident weights / tables ----
            # wq/wk/wv chunked along HT (4 x 8 h-tiles); DMAs are emitted in
            # 16 fine pieces (2 h-tiles each) on the scalar queue, just-in-
            # time interleaved with chunk 0's x stream, so the first matmul
            # only waits for piece 0 and the HBM isn't clogged at t=0.
            HC = HT // 4
            wq_sb = [persist.tile([128, HC, QD], BF16, name=f"wq{c}")
                     for c in range(4)]
            wk_sb = [persist.tile([128, HC, HD], BF16, name=f"wk{c}")
                     for c in range(4)]
            wv_sb = [persist.tile([128, HC, HD], BF16, name=f"wv{c}")
                     for c in range(4)]

            def load_w_piece(i):
                # piece i covers h-tiles 2i, 2i+1 -> chunk c=i//4, a-pair
                c, ap_ = divmod(i, 4)
                r = slice((c * HC + ap_ * 2) * 128, (c * HC + ap_ * 2 + 2) * 128)
                asl = slice(ap_ * 2, ap_ * 2 + 2)
                nc.scalar.dma_start(
                    wk_sb[c][:, asl, :],
                    wk[r, :].rearrange("(a p) q -> p a q", p=128))
                nc.scalar.dma_start(
                    wv_sb[c][:, asl, :],
                    wv[r, :].rearrange("(a p) q -> p a q", p=128))
                nc.scalar.dma_start(
                    wq_sb[c][:, asl, :],
                    wq[r, :].rearrange("(a p) q -> p a q", p=128))

            cos_sb = persist.tile([HD, S], BF16)
            ssin_sb = persist.tile([HD, S], BF16)
            ident_sb = persist.tile([128, 128], BF16)
            ones_sb = persist.tile([128, 128], BF16)
            wo_sb = persist.tile([128, QH_PER_CORE, H], BF16)
            for _i in range(4):
                load_w_piece(_i)

            def load_wo_part(i):
                # 16 x 256KB pieces: a 1MB burst here starves the x stream
                # (chunk-1 starvation seen at 60-80us in the v4 trace)
                od, j = divmod(i, 4)
                nc.scalar.dma_start(
                    wo_sb[:, od, j * 1024:(j + 1) * 1024],
                    wo[od * 128:(od + 1) * 128, j * 1024:(j + 1) * 1024])

            pend_w = []   # pending o-proj units, carried across batches
            pend_q = []   # deferred chunk-3 q-proj granules (per batch)
            cur = {}      # current batch's pools/staging

            def emit_w_unit(unit, tail=False):
                ots, ut0, ich, hch, tt = unit
                isl0 = ich * TCH
                key = (ut0, ich, hch)
                if key not in cur["oout"]:
                    cur["oout"][key] = cur["owp"].tile(
                        [128, 4, TCH], BF16, name="oout", tag="oout")
                w_ps = cur["wps"].tile([128, TCH], F32, name="wops", tag="wops")
                for od in range(QH_PER_CORE):
                    nc.tensor.matmul(
                        w_ps[:],
                        ots[od][ich][:, tt * 128:(tt + 1) * 128],
                        wo_sb[:, od, hch * TCH:(hch + 1) * TCH],
                        start=(od == 0), stop=(od == QH_PER_CORE - 1))
                ob = cur["oout"][key]
                # in the kernel tail ScalarE is exp-free, so splitting the
                # evacuations across both engines halves the drain latency
                if tail and tt % 2 == 0:
                    nc.scalar.copy(ob[:, tt, :], w_ps[:])
                else:
                    nc.vector.tensor_copy(ob[:, tt, :], w_ps[:])
                if tt == 3:
                    nc.gpsimd.dma_start(
                        out[ut0 + isl0:ut0 + isl0 + TCH,
                            hch * TCH:(hch + 1) * TCH]
                        .rearrange("(a p) hh -> p a hh", p=128),
                        ob[:])
                    del cur["oout"][key]

            def emit_q_granule(g):
                # deferred q-proj for chunk 3: one x tile (4 h-tiles), one
                # head, 4 accumulating matmuls into the single qfill bank.
                qh, i, tc0 = g
                x_t = cur["qx"].tile([128, 4, TCH], BF16, name="qxs", tag="qxs")
                nc.sync.dma_start(
                    x_t[:],
                    xt[i * 512:(i + 1) * 512, tc0:tc0 + TCH]
                    .rearrange("(a p) t -> p a t", p=128))
                if i == 0:
                    cur["qfps"] = cur["qps"].tile([128, TCH], F32, name="qfill",
                                                  tag="qfill")
                q_ps = cur["qfps"]
                for hi in range(4):
                    h = i * 4 + hi
                    hc, ho = divmod(h, HC)
                    nc.tensor.matmul(
                        q_ps[:],
                        wq_sb[hc][:, ho, qh * 128:(qh + 1) * 128],
                        x_t[:, hi, :], start=(h == 0), stop=(h == HT - 1))
                if i == 7:
                    # evac + rope into qt_sb[qh][3] -- all on DVE: the
                    # ScalarE is exp-saturated during the warmup and a
                    # scalar evac here would stall the next head's PSUM
                    # reuse (and the in-order PE queue behind it).
                    qt_dst, csl = cur["qdst"](qh)
                    raw = cur["qtmp"].tile([128, TCH], BF16, name="qfr",
                                           tag="qfr", bufs=1)
                    rsw = cur["qtmp"].tile([128, TCH], BF16, name="qfs",
                                           tag="qfs", bufs=1)
                    nc.vector.tensor_copy(raw[:], q_ps[:])
                    nc.vector.tensor_copy(rsw[0:64, :], q_ps[64:128, :])
                    nc.vector.tensor_copy(rsw[64:128, :], q_ps[0:64, :])
                    tA = cur["qtmp"].tile([128, TCH], BF16, name="qfA",
                                          tag="qfA", bufs=1)
                    tB = cur["qtmp"].tile([128, TCH], BF16, name="qfB",
                                          tag="qfB", bufs=1)
                    nc.vector.tensor_tensor(tA[:], raw[:], cos_sb[:, csl],
                                            mybir.AluOpType.mult)
                    nc.vector.tensor_tensor(tB[:], rsw[:], ssin_sb[:, csl],
                                            mybir.AluOpType.mult)
                    nc.vector.tensor_tensor(qt_dst, tA[:], tB[:],
                                            mybir.AluOpType.add)

            drain_tick = [0]

            def drain(k):
                # PE filler: deferred q-proj granules on every third slot
                # (paced to the ~1.4us/granule DMA rate -- over-eager
                # emission puts DMA-gated matmuls ahead of ready score
                # matmuls in the in-order PE queue), o-proj units on the
                # remaining slots.
                for _ in range(k):
                    drain_tick[0] += 1
                    if pend_q and drain_tick[0] % 3 == 0:
                        emit_q_granule(pend_q.pop(0))
                    elif pend_w:
                        emit_w_unit(pend_w.pop(0))

            def drain_w_only(k, tail=False):
                for _ in range(k):
                    if pend_w:
                        emit_w_unit(pend_w.pop(0), tail=tail)

            for b in range(B):
                t0 = b * S
                # per-batch activation stores, chunk-granular tiles
                qt_sb = [
                    [spool.tile([128, TCH], BF16, name=f"qt{qh}c{t}_b{b}",
                                tag=f"qt{qh}c{t}") for t in range(NTCH)]
                    for qh in range(QH_PER_CORE)
                ]
                kt_sb = [
                    spool.tile([128, TCH], BF16, name=f"ktc{t}_b{b}",
                               tag=f"ktc{t}") for t in range(NTCH)
                ]
                v_sb = spool.tile([128, JT, 128], BF16, name=f"v_b{b}", tag="v")
                ot_sb = [
                    [spool.tile([128, TCH], BF16, name=f"ot{qh}c{t}_b{b}",
                                tag=f"ot{qh}c{t}") for t in range(NTCH)]
                    for qh in range(QH_PER_CORE)
                ]

                # apool is kernel-level so the pre-computed score tiles of
                # the first two attention units (emitted during chunk 3's
                # K/V pass) survive into the A scope
                preA = {}

                def mk_evac(pool):
                    def evac(src_ps, raw_tag, eng):
                        raw = pool.tile([128, TCH], BF16, name=raw_tag,
                                        tag=raw_tag)
                        rsw = pool.tile([128, TCH], BF16, name=raw_tag + "s",
                                        tag=raw_tag + "s")
                        cp = nc.scalar.copy if eng == 0 else \
                            nc.vector.tensor_copy
                        cp(raw[:], src_ps[:])
                        cp(rsw[0:64, :], src_ps[64:128, :])
                        cp(rsw[64:128, :], src_ps[0:64, :])
                        return raw, rsw
                    return evac

                def mk_rope(pool, csl):
                    def rope_tt(dst, raw, rsw):
                        tA = pool.tile([128, TCH], BF16, name="ropeA",
                                       tag="ropeA")
                        nc.vector.tensor_tensor(
                            tA[:], raw[:], cos_sb[:, csl],
                            mybir.AluOpType.mult)
                        tB = pool.tile([128, TCH], BF16, name="ropeB",
                                       tag="ropeB")
                        nc.vector.tensor_tensor(
                            tB[:], rsw[:], ssin_sb[:, csl],
                            mybir.AluOpType.mult)
                        nc.vector.tensor_tensor(
                            dst, tA[:], tB[:], mybir.AluOpType.add)
                    return rope_tt

                # ---------------- P: QKV projections ----------------
                # chunks 0-2: full QKV.  chunk 3 (separate reduced scope
                # below): K/V only -- its Q matmuls are deferred into the A
                # loop as PE filler (pend_q).
                with tc.tile_pool(name="px", bufs=4) as px, \
                     tc.tile_pool(name="pt", bufs=2) as ptmp, \
                     tc.tile_pool(name="pps", bufs=1, space="PSUM") as pps, \
                     tc.tile_pool(name="vps", bufs=1, space="PSUM") as vps:
                    for tch in range(NTCH - 1):
                        tc0 = t0 + tch * TCH
                        q_ps = [
                            pps.tile([128, TCH], F32, name=f"qps{i}",
                                     tag=f"qps{i}")
                            for i in range(QH_PER_CORE)
                        ]
                        k_ps = pps.tile([128, TCH], F32, name="kps", tag="kps")
                        v_ps = pps.tile([128, TCH], F32, name="vps0", tag="vps0")
                        for hg in range(HT // 2):
                            x_t = px.tile([128, 2, TCH], BF16, name="xs", tag="xs")
                            nc.sync.dma_start(
                                x_t[:],
                                xt[hg * 256:(hg + 1) * 256, tc0:tc0 + TCH]
                                .rearrange("(a p) t -> p a t", p=128))
                            if b == 0 and tch == 0:
                                if hg < 12:
                                    load_w_piece(hg + 4)
                                elif hg == 12:
                                    # tiny; ident gates chunk-0 v-transpose
                                    nc.scalar.dma_start(ident_sb[:], ident[:])
                                    nc.scalar.dma_start(ones_sb[:], ones[:])
                                elif hg == 13:
                                    nc.scalar.dma_start(cos_sb[:], cosx[:])
                                elif hg == 14:
                                    nc.scalar.dma_start(ssin_sb[:], ssin[:])
                            if b == 0 and tch in (1, 2) and hg % 2 == 0:
                                load_wo_part((tch - 1) * 8 + hg // 2)
                            for hi in range(2):
                                h = hg * 2 + hi
                                hc, ho = divmod(h, HC)
                                first, last = h == 0, h == HT - 1
                                nc.tensor.matmul(k_ps[:], wk_sb[hc][:, ho, :],
                                                 x_t[:, hi, :], start=first,
                                                 stop=last)
                                nc.tensor.matmul(v_ps[:], wv_sb[hc][:, ho, :],
                                                 x_t[:, hi, :], start=first,
                                                 stop=last)
                                for qd in range(QH_PER_CORE):
                                    nc.tensor.matmul(
                                        q_ps[qd][:],
                                        wq_sb[hc][:, ho,
                                                  qd * 128:(qd + 1) * 128],
                                        x_t[:, hi, :], start=first,
                                        stop=last)
                        # evacuate PSUM via ScalarE to bf16, RoPE on DVE in
                        # 2x bf16 mode.  K first (it gates the A phase).
                        evac = mk_evac(ptmp)
                        rope_tt = mk_rope(ptmp,
                                          slice(tch * TCH, (tch + 1) * TCH))
                        vraw = ptmp.tile([128, TCH], BF16, name="vraw",
                                         tag="vraw")
                        kr = evac(k_ps, "kraw", 0)
                        nc.scalar.copy(vraw[:], v_ps[:])
                        rope_tt(kt_sb[tch][:], *kr)
                        q0r = evac(q_ps[0], "q0raw", 1)
                        q1r = evac(q_ps[1], "q1raw", 0)
                        q2r = evac(q_ps[2], "q2raw", 1)
                        q3r = evac(q_ps[3], "q3raw", 0)
                        rope_tt(qt_sb[0][tch][:], *q0r)
                        rope_tt(qt_sb[1][tch][:], *q1r)
                        rope_tt(qt_sb[2][tch][:], *q2r)
                        rope_tt(qt_sb[3][tch][:], *q3r)
                        for tt in range(TCH // 128):
                            vt_ps = vps.tile([128, 128], BF16, name="vtp",
                                             tag="vtp")
                            nc.tensor.transpose(
                                vt_ps[:], vraw[:, tt * 128:(tt + 1) * 128],
                                ident_sb[:])
                            nc.vector.tensor_copy(
                                v_sb[:, tch * 4 + tt, :], vt_ps[:])

                # ------- chunk 3: K/V only + pre-scores for units 0,1 -------
                # The first A units are exp-latency bound with an empty
                # drain queue; computing their first 12 j-tiles of scores
                # (which only need kt chunks 0-2) here fills both engines.
                with tc.tile_pool(name="px3", bufs=5) as px3, \
                     tc.tile_pool(name="pt3", bufs=2) as pt3, \
                     tc.tile_pool(name="pps3", bufs=1, space="PSUM") as pps3, \
                     tc.tile_pool(name="psps", bufs=2, space="PSUM") as psps:
                    tch = NTCH - 1
                    tc0 = t0 + tch * TCH
                    k_ps = pps3.tile([128, TCH], F32, name="kps", tag="kps")
                    v_ps = pps3.tile([128, TCH], F32, name="vps0", tag="vps0")
                    for u in (0, 1):
                        p_sb_u = apool.tile([128, JT, TCH], BF16,
                                            name=f"ptil_pre{u}", tag="ptil")
                        l1a_u = apool.tile([128, 4, TCH], BF16,
                                           name=f"l1a_pre{u}", tag="l1a",
                                           bufs=2)
                        preA[u] = (p_sb_u, l1a_u)

                    def pre_sc(u, jt):
                        p_sb_u, l1a_u = preA[u]
                        st_ps = psps.tile([128, TCH], F32, name="pst",
                                          tag="pst")
                        nc.tensor.matmul(
                            st_ps[:],
                            kt_sb[jt // 4][:, (jt % 4) * 128:
                                           (jt % 4 + 1) * 128],
                            qt_sb[u][0][:], start=True, stop=True)
                        nc.scalar.activation(
                            p_sb_u[:, jt, :], st_ps[:],
                            mybir.ActivationFunctionType.Exp,
                            scale=SCALE)
                        if jt == 7:
                            nc.vector.tensor_tensor(
                                l1a_u[:], p_sb_u[:, 0:4, :], p_sb_u[:, 4:8, :],
                                mybir.AluOpType.add)

                    pre_jobs = [(u, jt) for u in (0, 1) for jt in range(12)]
                    for hg in range(HT // 2):
                        x_t = px3.tile([128, 2, TCH], BF16, name="xs3",
                                       tag="xs3")
                        nc.sync.dma_start(
                            x_t[:],
                            xt[hg * 256:(hg + 1) * 256, tc0:tc0 + TCH]
                            .rearrange("(a p) t -> p a t", p=128))
                        for hi in range(2):
                            h = hg * 2 + hi
                            hc, ho = divmod(h, HC)
                            first, last = h == 0, h == HT - 1
                            nc.tensor.matmul(k_ps[:], wk_sb[hc][:, ho, :],
                                             x_t[:, hi, :], start=first,
                                             stop=last)
                            nc.tensor.matmul(v_ps[:], wv_sb[hc][:, ho, :],
                                             x_t[:, hi, :], start=first,
                                             stop=last)
                        for j in range(hg * 3 // 2, (hg + 1) * 3 // 2):
                            if j < len(pre_jobs):
                                pre_sc(*pre_jobs[j])
                    evac = mk_evac(pt3)
                    rope_tt = mk_rope(pt3, slice(tch * TCH, (tch + 1) * TCH))
                    vraw = pt3.tile([128, TCH], BF16, name="vraw", tag="vraw")
                    kr = evac(k_ps, "kraw", 0)
                    nc.scalar.copy(vraw[:], v_ps[:])
                    rope_tt(kt_sb[tch][:], *kr)
                    for tt in range(TCH // 128):
                        vt_ps = pps3.tile([128, 128], BF16, name="vtp",
                                          tag="vtp")
                        nc.tensor.transpose(
                            vt_ps[:], vraw[:, tt * 128:(tt + 1) * 128],
                            ident_sb[:])
                        nc.vector.tensor_copy(
                            v_sb[:, tch * 4 + tt, :], vt_ps[:])

                # ---------------- A + W (+ deferred q3) fused ----------------
                with tc.tile_pool(name="an", bufs=2) as anorm, \
                     tc.tile_pool(name="ow", bufs=2) as owp, \
                     tc.tile_pool(name="qx", bufs=3) as qxp, \
                     tc.tile_pool(name="qt2", bufs=1) as qtmp, \
                     tc.tile_pool(name="sps", bufs=2, space="PSUM") as sps, \
                     tc.tile_pool(name="ops", bufs=2, space="PSUM") as ops_, \
                     tc.tile_pool(name="dps", bufs=1, space="PSUM") as dps, \
                     tc.tile_pool(name="wps", bufs=2, space="PSUM") as wps, \
                     tc.tile_pool(name="qps", bufs=1, space="PSUM") as qpsp:
                    cur["owp"] = owp
                    cur["wps"] = wps
                    cur["oout"] = {}
                    cur["qx"] = qxp
                    cur["qps"] = qpsp
                    cur["qtmp"] = qtmp
                    qcsl = slice(3 * TCH, 4 * TCH)
                    cur["qdst"] = lambda qh: (qt_sb[qh][3][:], qcsl)
                    # deferred chunk-3 q-proj granules for THIS batch
                    tq0 = t0 + 3 * TCH
                    pend_q.extend((qh, i, tq0)
                                  for qh in range(QH_PER_CORE)
                                  for i in range(8))

                    def finish_prev(prev):
                        # evac PV output; rest of the den tree; broadcast-den
                        # matmul (all-ones stationary); fast reciprocal;
                        # per-head normalize.  On the last head, queue the
                        # i-chunk's o-proj units.
                        pich, pqh, p_sb, o_ps, l1a = prev
                        orw = apool.tile([128, TCH], BF16, name="oraw",
                                         tag=f"oraw{pqh}", bufs=1)
                        nc.scalar.copy(orw[:], o_ps[:])
                        # tree: l1a (issued mid-unit) holds jt 0-7; fold in
                        # jt 8-15, then reduce 8->4->2->1
                        l1b = qtmp.tile([128, 4, TCH], BF16, name="l1b",
                                        tag="l1b", bufs=1)
                        nc.vector.tensor_tensor(
                            l1b[:], p_sb[:, 8:12, :], p_sb[:, 12:16, :],
                            mybir.AluOpType.add)
                        nc.vector.tensor_tensor(
                            l1a[:], l1a[:], l1b[:], mybir.AluOpType.add)
                        nc.vector.tensor_tensor(
                            l1a[:, 0:2, :], l1a[:, 0:2, :], l1a[:, 2:4, :],
                            mybir.AluOpType.add)
                        t1 = anorm.tile([128, TCH], BF16, name="t1", tag="t1",
                                        bufs=2)
                        nc.vector.tensor_tensor(
                            t1[:], l1a[:, 0, :], l1a[:, 1, :],
                            mybir.AluOpType.add)
                        den_ps = dps.tile([128, TCH], F32, name="den",
                                          tag="den")
                        nc.tensor.matmul(den_ps[:], ones_sb[:], t1[:],
                                         start=True, stop=True)
                        rec = anorm.tile([128, TCH], F32, name="rec",
                                         tag="rec", bufs=1)
                        nc.vector.reciprocal_approx_fast(rec[:], den_ps[:])
                        nc.vector.tensor_tensor(
                            ot_sb[pqh][pich][:], rec[:], orw[:],
                            mybir.AluOpType.mult)
                        if pqh == QH_PER_CORE - 1:
                            pend_w.extend(
                                (ot_sb, t0, pich, hch, tt)
                                for hch in range(NHCH) for tt in range(4))

                    prev = None
                    for n in range(NTCH * QH_PER_CORE):
                        ich, qh = divmod(n, QH_PER_CORE)
                        if n in preA:
                            p_sb, l1a = preA.pop(n)
                            pre_done = 12   # j-tiles 0-11 already exp'd
                        else:
                            p_sb = apool.tile([128, JT, TCH], BF16,
                                              name="ptil", tag="ptil")
                            l1a = apool.tile([128, 4, TCH], BF16, name="l1a",
                                             tag="l1a", bufs=2)
                            pre_done = 0

                        def sc(jt):
                            if jt < pre_done:
                                return
                            st_ps = sps.tile([128, TCH], F32, name="st",
                                             tag="st")
                            nc.tensor.matmul(
                                st_ps[:],
                                kt_sb[jt // 4][:, (jt % 4) * 128:
                                               (jt % 4 + 1) * 128],
                                qt_sb[qh][ich][:], start=True, stop=True)
                            nc.scalar.activation(
                                p_sb[:, jt, :], st_ps[:],
                                mybir.ActivationFunctionType.Exp,
                                scale=SCALE)

                        def pv(jt):
                            if prev is None:
                                return
                            _, _, pp_sb, po_ps, _ = prev
                            nc.tensor.matmul(
                                po_ps[:], v_sb[:, jt, :], pp_sb[:, jt, :],
                                start=(jt == 0), stop=(jt == JT - 1))

                        for g in range(4):
                            jb = g * 4
                            sc(jb); sc(jb + 1)
                            pv(jb); pv(jb + 1)
                            drain(1)
                            sc(jb + 2); pv(jb + 2)
                            drain(1)
                            sc(jb + 3); pv(jb + 3)
                            drain(1)
                            if g == 1 and pre_done == 0:
                                # first half of this unit's den tree
                                nc.vector.tensor_tensor(
                                    l1a[:], p_sb[:, 0:4, :], p_sb[:, 4:8, :],
                                    mybir.AluOpType.add)
                        if prev is not None:
                            finish_prev(prev)
                        o_ps_n = ops_.tile([128, TCH], F32, name="opv",
                                           tag="opv")
                        prev = (ich, qh, p_sb, o_ps_n, l1a)

                    # tail: PV of the last head with 8 o-proj units drained
                    # to cover the den-tree latency; finish; carry the rest
                    # of the last i-chunk's o-proj into the next batch's
                    # warmup (or drain fully at kernel end).
                    _, _, pp_sb, po_ps, _ = prev
                    for jt in range(JT):
                        nc.tensor.matmul(po_ps[:], v_sb[:, jt, :],
                                         pp_sb[:, jt, :], start=(jt == 0),
                                         stop=(jt == JT - 1))
                        if jt % 4 == 3:
                            drain_w_only(2)
                    finish_prev(prev)
                    if b == B - 1:
                        drain_w_only(len(pend_w), tail=True)
    nc.finalize()
    return nc


_NC_CACHE = None


def _get_nc():
    global _NC_CACHE
    if _NC_CACHE is None:
        _NC_CACHE = _build_nc()
    return _NC_CACHE


def _host_tables():
    inv = 1.0 / (ROPE_BASE ** (np.arange(0, HD, 2, dtype=np.float64) / HD))
    t = np.arange(S, dtype=np.float64)
    freqs = np.outer(t, inv)                      # [S, 64]
    emb = np.concatenate([freqs, freqs], axis=1)  # [S, 128]
    cos = np.cos(emb).astype(np.float32).T.copy()   # [128, S]
    sin = np.sin(emb).astype(np.float32).T.copy()
    ssin = sin.copy()
    ssin[0:64, :] *= -1.0
    return np.ascontiguousarray(cos), np.ascontiguousarray(ssin)


def kernel(hidden_states, Wq, Wk, Wv, Wo, trace=False):
    import ml_dtypes
    BF = ml_dtypes.bfloat16

    hs = np.asarray(hidden_states, dtype=np.float32)
    Wq = np.asarray(Wq, dtype=np.float32)
    Wk = np.asarray(Wk, dtype=np.float32)
    Wv = np.asarray(Wv, dtype=np.float32)
    Wo = np.asarray(Wo, dtype=np.float32)

    xt = np.ascontiguousarray(hs.reshape(T, H).T).astype(BF)   # [H, T]
    cos, ssin = _host_tables()
    cos_bf = cos.astype(BF)
    ssin_bf = ssin.astype(BF)
    ident = np.eye(128, dtype=BF)
    ones = np.ones((128, 128), dtype=BF)

    in_maps = []
    for c in range(NCORES):
        in_maps.append({
            "xt": xt,
            "wq": np.ascontiguousarray(Wq[c * QD:(c + 1) * QD, :].T).astype(BF),
            "wk": np.ascontiguousarray(Wk[c * HD:(c + 1) * HD, :].T).astype(BF),
            "wv": np.ascontiguousarray(Wv[c * HD:(c + 1) * HD, :].T).astype(BF),
            "wo": np.ascontiguousarray(Wo[:, c * QD:(c + 1) * QD].T).astype(BF),
            "cosx": cos_bf,
            "ssin": ssin_bf,
            "ident": ident,
            "ones": ones,
        })

    nc = _get_nc()
    res = run_bass_kernel_spmd(nc, in_maps, list(range(NCORES)), trace=trace)
    acc = np.zeros((T, H), dtype=np.float32)
    for c in range(NCORES):
        acc += res.results[c]["out_part"].astype(np.float32)
    out = acc.reshape(B, S, H)
    if trace:
        return out, res
    return out


# revision 26
# speedup vs baseline: 1.2547x; 1.2547x over previous
#!/usr/bin/env python3
"""GroupedQueryAttention Trainium2 kernel, tensor-parallel over heads on 8
NeuronCores.

Reference model: B=2, S=2048, H=4096, NH=32 query heads, NKV=8 kv heads,
HD=128, RoPE base 5e5, softmax attention, o-proj.

Sharding: core c owns kv head c and query heads 4c..4c+3 (groups stay
aligned).  Wq/Wo sharded by query head, Wk/Wv by kv head.  Each core
computes a rank-512 slice of the o-proj contraction; the host sums the 8
partial outputs (the all-reduce of row-parallel TP done on host at gather
time).

v4 design notes (on top of v3):
  * Weight loads split into 16 just-in-time pieces on the scalar queue so
    the first projection matmul starts ~5us in instead of ~28us; cos/sin/
    tables/wo trickle in behind the x stream.
  * Chunk 3's Q projection is deferred out of the P phase into the A loop
    as PE filler for the exp-latency-bound first attention units (its x
    tiles are re-streamed per head on the idle sync queue).  Combined with
    o-proj units carried across the batch boundary, the PE no longer
    idles (and HAM no longer drops to half clock) at phase transitions.
  * Softmax denominator: the per-head tree is non-destructive and its
    first level issues mid-unit; the cross-partition sum uses an all-ones
    [128,128] stationary so ONE matmul yields the denominator already
    broadcast to 128 partitions (the old per-head bcsel broadcast matmuls
    and their PSUM bank are gone -- that bank now feeds the Q filler).
    reciprocal_approx_fast (single custom-DVE op, ~18 bits) replaces the
    slow iterative reciprocal; normalize happens per head.
"""
import sys

for _p in ("/opt/trn_rl_repo",):
    if _p not in sys.path:
        sys.path.insert(0, _p)

import numpy as np

import concourse.bacc as bacc
import concourse.mybir as mybir
from concourse import tile
from concourse.bass_utils import run_bass_kernel_spmd

B, S, H = 2, 2048, 4096
NH, NKV, HD = 32, 8, 128
NCORES = 8
QH_PER_CORE = NH // NCORES          # 4 query heads / core
QD = QH_PER_CORE * HD               # 512 q dims / core
ROPE_BASE = 500000.0
T = B * S                           # 4096 tokens
TCH = 512                           # token chunk (proj N, attn i-chunk)
NTCH = S // TCH                     # 4 chunks per batch
HT = H // 128                       # 32 h-tiles
JT = S // 128                       # 16 j-tiles per batch
NHCH = H // TCH                     # 8 o-proj column chunks
SCALE = 1.0 / np.sqrt(HD)

F32 = mybir.dt.float32
BF16 = mybir.dt.bfloat16


def _build_nc():
    nc = bacc.Bacc("TRN2", target_bir_lowering=False, debug=False)
    xt = nc.dram_tensor("xt", [H, T], BF16, kind="ExternalInput").ap()
    wq = nc.dram_tensor("wq", [H, QD], BF16, kind="ExternalInput").ap()
    wk = nc.dram_tensor("wk", [H, HD], BF16, kind="ExternalInput").ap()
    wv = nc.dram_tensor("wv", [H, HD], BF16, kind="ExternalInput").ap()
    wo = nc.dram_tensor("wo", [QD, H], BF16, kind="ExternalInput").ap()
    cosx = nc.dram_tensor("cosx", [HD, S], BF16, kind="ExternalInput").ap()
    ssin = nc.dram_tensor("ssin", [HD, S], BF16, kind="ExternalInput").ap()
    ident = nc.dram_tensor("ident", [128, 128], BF16, kind="ExternalInput").ap()
    # all-ones stationary: one matmul = partition-sum broadcast to all rows
    ones = nc.dram_tensor("ones", [128, 128], BF16, kind="ExternalInput").ap()
    out = nc.dram_tensor("out_part", [T, H], BF16, kind="ExternalOutput").ap()

    with tile.TileContext(nc) as tc, \
         nc.allow_low_precision(reason="bf16 matmuls; bf16 attn probs"):
        with tc.tile_pool(name="persist", bufs=1) as persist, \
             tc.tile_pool(name="store", bufs=1) as spool:
            # ---- resident weights / tables ----
            # wq/wk/wv chunked along HT (4 x 8 h-tiles); DMAs are emitted in
            # 16 fine pieces (2 h-tiles each) on the scalar queue, just-in-
            # time interleaved with chunk 0's x stream, so the first matmul
            # only waits for piece 0 and the HBM isn't clogged at t=0.
            HC = HT // 4
            wq_sb = [persist.tile([128, HC, QD], BF16, name=f"wq{c}")
                     for c in range(4)]
            wk_sb = [persist.tile([128, HC, HD], BF16, name=f"wk{c}")
                     for c in range(4)]
            wv_sb = [persist.tile([128, HC, HD], BF16, name=f"wv{c}")
                     for c in range(4)]

            def load_w_half(c, half):
                # 4 h-tiles of chunk c (3 DMAs).  DMA COUNT matters: each
                # queue issue costs ~0.6-0.8us, so fine-grained pieces
                # serialize the scalar queue and starve the PE (seen as the
                # 0-80us dips in the v5 trace).
                r = slice((c * HC + half * 4) * 128,
                          (c * HC + half * 4 + 4) * 128)
                asl = slice(half * 4, half * 4 + 4)
                nc.scalar.dma_start(
                    wk_sb[c][:, asl, :],
                    wk[r, :].rearrange("(a p) q -> p a q", p=128))
                nc.scalar.dma_start(
                    wv_sb[c][:, asl, :],
                    wv[r, :].rearrange("(a p) q -> p a q", p=128))
                nc.scalar.dma_start(
                    wq_sb[c][:, asl, :],
                    wq[r, :].rearrange("(a p) q -> p a q", p=128))

            def load_w_chunk(c):
                load_w_half(c, 0)
                load_w_half(c, 1)

            cos_sb = persist.tile([HD, S], BF16)
            ssin_sb = persist.tile([HD, S], BF16)
            ident_sb = persist.tile([128, 128], BF16)
            ones_sb = persist.tile([128, 128], BF16)
            wo_sb = persist.tile([128, QH_PER_CORE, H], BF16)
            # tiny tables first (ident gates chunk-0's v-transpose and must
            # not queue behind megabytes of weights), then chunk-0 weights
            nc.scalar.dma_start(ident_sb[:], ident[:])
            nc.scalar.dma_start(ones_sb[:], ones[:])
            load_w_chunk(0)

            def load_wo_part(i):
                # 16 x 256KB pieces: a 1MB burst here starves the x stream
                # (chunk-1 starvation seen at 60-80us in the v4 trace)
                od, j = divmod(i, 4)
                nc.scalar.dma_start(
                    wo_sb[:, od, j * 1024:(j + 1) * 1024],
                    wo[od * 128:(od + 1) * 128, j * 1024:(j + 1) * 1024])

            pend_w = []   # pending o-proj units, carried across batches
            pend_q = []   # deferred chunk-3 q-proj granules (per batch)
            cur = {}      # current batch's pools/staging

            def emit_w_unit(unit, tail=False):
                ots, ut0, ich, hch, tt = unit
                isl0 = ich * TCH
                key = (ut0, ich, hch)
                if key not in cur["oout"]:
                    cur["oout"][key] = cur["owp"].tile(
                        [128, 4, TCH], BF16, name="oout", tag="oout")
                w_ps = cur["wps"].tile([128, TCH], F32, name="wops", tag="wops")
                for od in range(QH_PER_CORE):
                    nc.tensor.matmul(
                        w_ps[:],
                        ots[od][ich][:, tt * 128:(tt + 1) * 128],
                        wo_sb[:, od, hch * TCH:(hch + 1) * TCH],
                        start=(od == 0), stop=(od == QH_PER_CORE - 1))
                ob = cur["oout"][key]
                # in the kernel tail ScalarE is exp-free, so splitting the
                # evacuations across both engines halves the drain latency,
                # and per-tt output DMAs start the final writes sooner
                if tail and tt % 2 == 0:
                    nc.scalar.copy(ob[:, tt, :], w_ps[:])
                else:
                    nc.vector.tensor_copy(ob[:, tt, :], w_ps[:])
                if tail:
                    nc.gpsimd.dma_start(
                        out[ut0 + isl0 + tt * 128:ut0 + isl0 + (tt + 1) * 128,
                            hch * TCH:(hch + 1) * TCH],
                        ob[:, tt, :])
                    if tt == 3:
                        del cur["oout"][key]
                elif tt == 3:
                    nc.gpsimd.dma_start(
                        out[ut0 + isl0:ut0 + isl0 + TCH,
                            hch * TCH:(hch + 1) * TCH]
                        .rearrange("(a p) hh -> p a hh", p=128),
                        ob[:])
                    del cur["oout"][key]

            def emit_q_granule(g):
                # deferred q-proj for chunk 3: one x tile (4 h-tiles), one
                # head, 4 accumulating matmuls into the single qfill bank.
                qh, i, tc0 = g
                x_t = cur["qx"].tile([128, 4, TCH], BF16, name="qxs", tag="qxs")
                nc.sync.dma_start(
                    x_t[:],
                    xt[i * 512:(i + 1) * 512, tc0:tc0 + TCH]
                    .rearrange("(a p) t -> p a t", p=128))
                if i == 0:
                    cur["qfps"] = cur["qps"].tile([128, TCH], F32, name="qfill",
                                                  tag="qfill")
                q_ps = cur["qfps"]
                for hi in range(4):
                    h = i * 4 + hi
                    hc, ho = divmod(h, HC)
                    nc.tensor.matmul(
                        q_ps[:],
                        wq_sb[hc][:, ho, qh * 128:(qh + 1) * 128],
                        x_t[:, hi, :], start=(h == 0), stop=(h == HT - 1))
                if i == 7:
                    # evac + rope into qt_sb[qh][3] -- all on DVE: the
                    # ScalarE is exp-saturated during the warmup and a
                    # scalar evac here would stall the next head's PSUM
                    # reuse (and the in-order PE queue behind it).
                    qt_dst, csl = cur["qdst"](qh)
                    raw = cur["qtmp"].tile([128, TCH], BF16, name="qfr",
                                           tag="qfr", bufs=1)
                    rsw = cur["qtmp"].tile([128, TCH], BF16, name="qfs",
                                           tag="qfs", bufs=1)
                    nc.vector.tensor_copy(raw[:], q_ps[:])
                    nc.vector.tensor_copy(rsw[0:64, :], q_ps[64:128, :])
                    nc.vector.tensor_copy(rsw[64:128, :], q_ps[0:64, :])
                    tA = cur["qtmp"].tile([128, TCH], BF16, name="qfA",
                                          tag="qfA", bufs=1)
                    tB = cur["qtmp"].tile([128, TCH], BF16, name="qfB",
                                          tag="qfB", bufs=1)
                    nc.vector.tensor_tensor(tA[:], raw[:], cos_sb[:, csl],
                                            mybir.AluOpType.mult)
                    nc.vector.tensor_tensor(tB[:], rsw[:], ssin_sb[:, csl],
                                            mybir.AluOpType.mult)
                    nc.vector.tensor_tensor(qt_dst, tA[:], tB[:],
                                            mybir.AluOpType.add)

            drain_tick = [0]

            def drain(k):
                # PE filler: deferred q-proj granules on every third slot
                # (paced to the ~1.4us/granule DMA rate -- over-eager
                # emission puts DMA-gated matmuls ahead of ready score
                # matmuls in the in-order PE queue), o-proj units on the
                # remaining slots.
                for _ in range(k):
                    drain_tick[0] += 1
                    if pend_q and drain_tick[0] % 3 == 0:
                        emit_q_granule(pend_q.pop(0))
                    elif pend_w:
                        emit_w_unit(pend_w.pop(0))

            def drain_w_only(k, tail=False):
                for _ in range(k):
                    if pend_w:
                        emit_w_unit(pend_w.pop(0), tail=tail)

            for b in range(B):
                t0 = b * S
                # per-batch activation stores, chunk-granular tiles
                qt_sb = [
                    [spool.tile([128, TCH], BF16, name=f"qt{qh}c{t}_b{b}",
                                tag=f"qt{qh}c{t}") for t in range(NTCH)]
                    for qh in range(QH_PER_CORE)
                ]
                kt_sb = [
                    spool.tile([128, TCH], BF16, name=f"ktc{t}_b{b}",
                               tag=f"ktc{t}") for t in range(NTCH)
                ]
                v_sb = spool.tile([128, JT, 128], BF16, name=f"v_b{b}", tag="v")
                ot_sb = [
                    [spool.tile([128, TCH], BF16, name=f"ot{qh}c{t}_b{b}",
                                tag=f"ot{qh}c{t}") for t in range(NTCH)]
                    for qh in range(QH_PER_CORE)
                ]

                # ---------------- P: QKV projections ----------------
                # chunks 0-2: full QKV.  chunk 3: K/V only -- its Q matmuls
                # are deferred into the A loop as PE filler (pend_q).
                with tc.tile_pool(name="px", bufs=6) as px, \
                     tc.tile_pool(name="pt", bufs=2) as ptmp, \
                     tc.tile_pool(name="pps", bufs=1, space="PSUM") as pps, \
                     tc.tile_pool(name="vps", bufs=1, space="PSUM") as vps:
                    for tch in range(NTCH):
                        kv_only = tch == NTCH - 1
                        tc0 = t0 + tch * TCH
                        if not kv_only:
                            q_ps = [
                                pps.tile([128, TCH], F32, name=f"qps{i}",
                                         tag=f"qps{i}")
                                for i in range(QH_PER_CORE)
                            ]
                        k_ps = pps.tile([128, TCH], F32, name="kps", tag="kps")
                        v_ps = pps.tile([128, TCH], F32, name="vps0", tag="vps0")
                        for hg in range(HT // 2):
                            x_t = px.tile([128, 2, TCH], BF16, name="xs", tag="xs")
                            nc.sync.dma_start(
                                x_t[:],
                                xt[hg * 256:(hg + 1) * 256, tc0:tc0 + TCH]
                                .rearrange("(a p) t -> p a t", p=128))
                            if b == 0 and tch == 0:
                                if hg in (2, 6, 10):
                                    load_w_chunk(hg // 4 + 1)
                                elif hg == 13:
                                    nc.scalar.dma_start(cos_sb[:], cosx[:])
                                elif hg == 14:
                                    nc.scalar.dma_start(ssin_sb[:], ssin[:])
                            if b == 0 and tch in (1, 2) and hg % 2 == 0:
                                load_wo_part((tch - 1) * 8 + hg // 2)
                            for hi in range(2):
                                h = hg * 2 + hi
                                hc, ho = divmod(h, HC)
                                first, last = h == 0, h == HT - 1
                                nc.tensor.matmul(k_ps[:], wk_sb[hc][:, ho, :],
                                                 x_t[:, hi, :], start=first,
                                                 stop=last)
                                nc.tensor.matmul(v_ps[:], wv_sb[hc][:, ho, :],
                                                 x_t[:, hi, :], start=first,
                                                 stop=last)
                                if not kv_only:
                                    for qd in range(QH_PER_CORE):
                                        nc.tensor.matmul(
                                            q_ps[qd][:],
                                            wq_sb[hc][:, ho,
                                                      qd * 128:(qd + 1) * 128],
                                            x_t[:, hi, :], start=first,
                                            stop=last)
                        # evacuate PSUM via ScalarE to bf16, RoPE on DVE in
                        # 2x bf16 mode.  K first (it gates the A phase).
                        csl = slice(tch * TCH, (tch + 1) * TCH)

                        def evac(src_ps, raw_tag, eng):
                            raw = ptmp.tile([128, TCH], BF16, name=raw_tag,
                                            tag=raw_tag)
                            rsw = ptmp.tile([128, TCH], BF16,
                                            name=raw_tag + "s",
                                            tag=raw_tag + "s")
                            cp = nc.scalar.copy if eng == 0 else \
                                nc.vector.tensor_copy
                            cp(raw[:], src_ps[:])
                            cp(rsw[0:64, :], src_ps[64:128, :])
                            cp(rsw[64:128, :], src_ps[0:64, :])
                            return raw, rsw

                        def rope_tt(dst, raw, rsw):
                            tA = ptmp.tile([128, TCH], BF16, name="ropeA",
                                           tag="ropeA")
                            nc.vector.tensor_tensor(
                                tA[:], raw[:], cos_sb[:, csl],
                                mybir.AluOpType.mult)
                            tB = ptmp.tile([128, TCH], BF16, name="ropeB",
                                           tag="ropeB")
                            nc.vector.tensor_tensor(
                                tB[:], rsw[:], ssin_sb[:, csl],
                                mybir.AluOpType.mult)
                            nc.vector.tensor_tensor(
                                dst, tA[:], tB[:], mybir.AluOpType.add)

                        vraw = ptmp.tile([128, TCH], BF16, name="vraw",
                                         tag="vraw")
                        kr = evac(k_ps, "kraw", 0)
                        nc.scalar.copy(vraw[:], v_ps[:])
                        rope_tt(kt_sb[tch][:], *kr)
                        if not kv_only:
                            q0r = evac(q_ps[0], "q0raw", 1)
                            q1r = evac(q_ps[1], "q1raw", 0)
                            q2r = evac(q_ps[2], "q2raw", 1)
                            q3r = evac(q_ps[3], "q3raw", 0)
                            rope_tt(qt_sb[0][tch][:], *q0r)
                            rope_tt(qt_sb[1][tch][:], *q1r)
                            rope_tt(qt_sb[2][tch][:], *q2r)
                            rope_tt(qt_sb[3][tch][:], *q3r)
                        for tt in range(TCH // 128):
                            vt_ps = vps.tile([128, 128], BF16, name="vtp",
                                             tag="vtp")
                            nc.tensor.transpose(
                                vt_ps[:], vraw[:, tt * 128:(tt + 1) * 128],
                                ident_sb[:])
                            nc.vector.tensor_copy(
                                v_sb[:, tch * 4 + tt, :], vt_ps[:])

                # ---------------- A + W (+ deferred q3) fused ----------------
                with tc.tile_pool(name="ap", bufs=2) as apool, \
                     tc.tile_pool(name="an", bufs=2) as anorm, \
                     tc.tile_pool(name="ow", bufs=2) as owp, \
                     tc.tile_pool(name="qx", bufs=3) as qxp, \
                     tc.tile_pool(name="qt2", bufs=1) as qtmp, \
                     tc.tile_pool(name="sps", bufs=2, space="PSUM") as sps, \
                     tc.tile_pool(name="ops", bufs=2, space="PSUM") as ops_, \
                     tc.tile_pool(name="dps", bufs=1, space="PSUM") as dps, \
                     tc.tile_pool(name="wps", bufs=2, space="PSUM") as wps, \
                     tc.tile_pool(name="qps", bufs=1, space="PSUM") as qpsp:
                    cur["owp"] = owp
                    cur["wps"] = wps
                    cur["oout"] = {}
                    cur["qx"] = qxp
                    cur["qps"] = qpsp
                    cur["qtmp"] = qtmp
                    qcsl = slice(3 * TCH, 4 * TCH)
                    cur["qdst"] = lambda qh: (qt_sb[qh][3][:], qcsl)
                    # deferred chunk-3 q-proj granules for THIS batch
                    tq0 = t0 + 3 * TCH
                    pend_q.extend((qh, i, tq0)
                                  for qh in range(QH_PER_CORE)
                                  for i in range(8))

                    def finish_prev(prev):
                        # evac PV output; rest of the den tree; broadcast-den
                        # matmul (all-ones stationary); fast reciprocal;
                        # per-head normalize.  On the last head, queue the
                        # i-chunk's o-proj units.
                        pich, pqh, p_sb, o_ps, l1a = prev
                        orw = apool.tile([128, TCH], BF16, name="oraw",
                                         tag=f"oraw{pqh}", bufs=1)
                        nc.scalar.copy(orw[:], o_ps[:])
                        # tree: l1a (issued mid-unit) holds jt 0-7; fold in
                        # jt 8-15, then reduce 8->4->2->1
                        l1b = qtmp.tile([128, 4, TCH], BF16, name="l1b",
                                        tag="l1b", bufs=1)
                        nc.vector.tensor_tensor(
                            l1b[:], p_sb[:, 8:12, :], p_sb[:, 12:16, :],
                            mybir.AluOpType.add)
                        nc.vector.tensor_tensor(
                            l1a[:], l1a[:], l1b[:], mybir.AluOpType.add)
                        nc.vector.tensor_tensor(
                            l1a[:, 0:2, :], l1a[:, 0:2, :], l1a[:, 2:4, :],
                            mybir.AluOpType.add)
                        t1 = anorm.tile([128, TCH], BF16, name="t1", tag="t1",
                                        bufs=2)
                        nc.vector.tensor_tensor(
                            t1[:], l1a[:, 0, :], l1a[:, 1, :],
                            mybir.AluOpType.add)
                        den_ps = dps.tile([128, TCH], F32, name="den",
                                          tag="den")
                        nc.tensor.matmul(den_ps[:], ones_sb[:], t1[:],
                                         start=True, stop=True)
                        rec = anorm.tile([128, TCH], F32, name="rec",
                                         tag="rec", bufs=1)
                        nc.vector.reciprocal_approx_fast(rec[:], den_ps[:])
                        nc.vector.tensor_tensor(
                            ot_sb[pqh][pich][:], rec[:], orw[:],
                            mybir.AluOpType.mult)
                        if pqh == QH_PER_CORE - 1:
                            pend_w.extend(
                                (ot_sb, t0, pich, hch, tt)
                                for hch in range(NHCH) for tt in range(4))

                    prev = None
                    for n in range(NTCH * QH_PER_CORE):
                        ich, qh = divmod(n, QH_PER_CORE)
                        p_sb = apool.tile([128, JT, TCH], BF16, name="ptil",
                                          tag="ptil")
                        l1a = apool.tile([128, 4, TCH], BF16, name="l1a",
                                         tag="l1a", bufs=2)

                        def sc(jt):
                            st_ps = sps.tile([128, TCH], F32, name="st",
                                             tag="st")
                            nc.tensor.matmul(
                                st_ps[:],
                                kt_sb[jt // 4][:, (jt % 4) * 128:
                                               (jt % 4 + 1) * 128],
                                qt_sb[qh][ich][:], start=True, stop=True)
                            nc.scalar.activation(
                                p_sb[:, jt, :], st_ps[:],
                                mybir.ActivationFunctionType.Exp,
                                scale=SCALE)

                        def pv(jt):
                            if prev is None:
                                return
                            _, _, pp_sb, po_ps, _ = prev
                            nc.tensor.matmul(
                                po_ps[:], v_sb[:, jt, :], pp_sb[:, jt, :],
                                start=(jt == 0), stop=(jt == JT - 1))

                        for g in range(4):
                            jb = g * 4
                            sc(jb); sc(jb + 1)
                            pv(jb); pv(jb + 1)
                            drain(1)
                            sc(jb + 2); pv(jb + 2)
                            drain(1)
                            sc(jb + 3); pv(jb + 3)
                            drain(1)
                            if g == 1:
                                # first half of this unit's den tree
                                nc.vector.tensor_tensor(
                                    l1a[:], p_sb[:, 0:4, :], p_sb[:, 4:8, :],
                                    mybir.AluOpType.add)
                        if prev is not None:
                            finish_prev(prev)
                        o_ps_n = ops_.tile([128, TCH], F32, name="opv",
                                           tag="opv")
                        prev = (ich, qh, p_sb, o_ps_n, l1a)

                    # tail: PV of the last head with 8 o-proj units drained
                    # to cover the den-tree latency; finish; carry the rest
                    # of the last i-chunk's o-proj into the next batch's
                    # warmup (or drain fully at kernel end).
                    _, _, pp_sb, po_ps, _ = prev
                    for jt in range(JT):
                        nc.tensor.matmul(po_ps[:], v_sb[:, jt, :],
                                         pp_sb[:, jt, :], start=(jt == 0),
                                         stop=(jt == JT - 1))
                        if jt % 4 == 3:
                            drain_w_only(2)
                    finish_prev(prev)
                    if b == B - 1:
                        drain_w_only(len(pend_w), tail=True)
    nc.finalize()
    return nc


_NC_CACHE = None


def _get_nc():
    global _NC_CACHE
    if _NC_CACHE is None:
        _NC_CACHE = _build_nc()
    return _NC_CACHE


def _host_tables():
    inv = 1.0 / (ROPE_BASE ** (np.arange(0, HD, 2, dtype=np.float64) / HD))
    t = np.arange(S, dtype=np.float64)
    freqs = np.outer(t, inv)                      # [S, 64]
    emb = np.concatenate([freqs, freqs], axis=1)  # [S, 128]
    cos = np.cos(emb).astype(np.float32).T.copy()   # [128, S]
    sin = np.sin(emb).astype(np.float32).T.copy()
    ssin = sin.copy()
    ssin[0:64, :] *= -1.0
    return np.ascontiguousarray(cos), np.ascontiguousarray(ssin)


def kernel(hidden_states, Wq, Wk, Wv, Wo, trace=False):
    import ml_dtypes
    BF = ml_dtypes.bfloat16

    hs = np.asarray(hidden_states, dtype=np.float32)
    Wq = np.asarray(Wq, dtype=np.float32)
    Wk = np.asarray(Wk, dtype=np.float32)
    Wv = np.asarray(Wv, dtype=np.float32)
    Wo = np.asarray(Wo, dtype=np.float32)

    xt = np.ascontiguousarray(hs.reshape(T, H).T).astype(BF)   # [H, T]
    cos, ssin = _host_tables()
    cos_bf = cos.astype(BF)
    ssin_bf = ssin.astype(BF)
    ident = np.eye(128, dtype=BF)
    ones = np.ones((128, 128), dtype=BF)

    in_maps = []
    for c in range(NCORES):
        in_maps.append({
            "xt": xt,
            "wq": np.ascontiguousarray(Wq[c * QD:(c + 1) * QD, :].T).astype(BF),
            "wk": np.ascontiguousarray(Wk[c * HD:(c + 1) * HD, :].T).astype(BF),
            "wv": np.ascontiguousarray(Wv[c * HD:(c + 1) * HD, :].T).astype(BF),
            "wo": np.ascontiguousarray(Wo[:, c * QD:(c + 1) * QD].T).astype(BF),
            "cosx": cos_bf,
            "ssin": ssin_bf,
            "ident": ident,
            "ones": ones,
        })

    nc = _get_nc()
    res = run_bass_kernel_spmd(nc, in_maps, list(range(NCORES)), trace=trace)
    acc = np.zeros((T, H), dtype=np.float32)
    for c in range(NCORES):
        acc += res.results[c]["out_part"].astype(np.float32)
    out = acc.reshape(B, S, H)
    if trace:
        return out, res
    return out
